# revision 4
# baseline (speedup 1.0000x reference)
"""Trainium2 Bass kernel for the neural-ODE (SEIR + neural hidden state) problem.

Strategy (single trajectory, strictly sequential RK4):
  - Runs on 1 of the 8 NeuronCores (program replicated SPMD; core 0's result used).
  - All weights resident in SBUF; per RHS eval the MLP matvecs run on the
    TensorEngine in "Mapping B": activation vector z is the (tiny) stationary
    operand [128,1] per k-tile, W^T streams as the moving operand in fp32r
    (reduced-precision fp32, ~2^-14 rel err) accumulating in PSUM.
  - softplus = ln(1+exp(p)) via the natural_log_exp ACT table set (loaded once,
    never switched).  Layer bias folded multiplicatively: exp(p+b)=exp(p)*exp(b)
    with exp(b) a host-precomputed constant applied after the PE transpose.
  - tanh / sigmoid computed as DVE odd polynomials (their inputs are small).
  - 17-dim SEIR ODE evaluated with tiny PE matmuls:
      dstate = AE^T @ [state;1] + D^T @ (P*state),  P = bb1*I broadcast by PE.
  - Time loop: tc.For_i over 63 intervals, 8 RK4 substeps statically unrolled
    inside; dt baked as immediates (requires uniform ts spacing, which
    setup_inputs provides).  Trajectory rows collected in an SBUF shift
    register; one DMA at the end.
"""
import sys
sys.path.insert(0, '/opt/trn_rl_repo')
import numpy as np

HIDDEN = 256
WIDTH = 1024
DEPTH = 4
T = 64
SUBSTEPS = 8
NSTATE = 17

# ---------------------------------------------------------------------------
# SEIR matrices:  dstate = c0 + A @ state + P * (D @ state),  P = bb1 * I
# ---------------------------------------------------------------------------
def seir_matrices():
    xi, mu, sigma, nu, gamma = 13 / 12, 0.041 / 12, 91 / 12, 36 / 12, 1.8 / 12
    A = np.zeros((17, 17), dtype=np.float64)
    D = np.zeros((17, 17), dtype=np.float64)
    c0 = np.zeros(17, dtype=np.float64)
    # indices: 0 M, 1 S1, 2 E1, 3 E2, 4 E3, 5 E4, 6 I1, 7 I2, 8 I3, 9 I4,
    #          10 R1, 11 R2, 12 R3, 13 R4, 14 S2, 15 S3, 16 S4
    A[0, 10:14] = mu; A[0, 0] = -(xi + mu)
    c0[1] = mu; A[1, 10:14] = -mu; A[1, 0] = xi; A[1, 1] = -mu; D[1, 1] = -1.0
    A[2, 2] = -(mu + sigma); D[2, 1] = 1.0
    A[3, 3] = -(mu + sigma); D[3, 14] = 0.5
    A[4, 4] = -(mu + sigma); D[4, 15] = 0.35
    A[5, 5] = -(mu + sigma); D[5, 16] = 0.25
    for j in range(4):
        A[6 + j, 2 + j] = sigma; A[6 + j, 6 + j] = -(nu + mu)
        A[10 + j, 6 + j] = nu; A[10 + j, 10 + j] = -(mu + gamma)
    A[14, 10] = gamma; A[14, 14] = -mu; D[14, 14] = -0.5
    A[15, 11] = gamma; A[15, 15] = -mu; D[15, 15] = -0.35
    A[16, 12] = gamma; A[16, 13] = gamma; A[16, 16] = -mu; D[16, 16] = -0.25
    return A, D, c0

# tanh odd-polynomial coefficients (Taylor; inputs stay well inside |u|<0.7)
TC1, TC3, TC5, TC7 = 1.0, -1.0 / 3.0, 2.0 / 15.0, -17.0 / 315.0


def _softmax32(x):
    x = np.asarray(x, np.float32)
    e = np.exp(x - x.max())
    return (e / e.sum()).astype(np.float32)


# ---------------------------------------------------------------------------
# Host-side packing of every constant the device needs, as one fp32 blob.
# Layout is a dict of (offset, shape) regions over a [128, NB] array.
# ---------------------------------------------------------------------------
def pack_blob(W0, b0, Wh, bh, Wl, bl, beta_W, beta_b, hvec, scale, y0_log):
    regions = {}
    cols = [0]

    def reg(name, ncols):
        off = cols[0]
        regions[name] = (off, ncols)
        cols[0] += ncols
        return off

    reg("W0T", 3 * WIDTH)
    for l in range(DEPTH - 1):
        reg(f"WhT{l}", 8 * WIDTH)
    reg("WlT", 8 * HIDDEN)
    reg("bI", 6)
    reg("AE", NSTATE)
    reg("DE", NSTATE)
    reg("ones17", NSTATE)
    for l in range(DEPTH - 1):
        reg(f"Eb{l}", 8)
    reg("blT", 2)
    reg("y0", 3)
    reg("onef", 1)
    NB = cols[0]

    blob = np.zeros((128, NB), dtype=np.float32)

    def put(name, arr):
        off, nc_ = regions[name]
        arr = np.asarray(arr, dtype=np.float32)
        blob[: arr.shape[0], off: off + arr.shape[1]] = arr

    # W0T: tiles k0,k1 = W0.T rows 0:128, 128:256 ; tile2 rows0:17 = W0.T[256:273],
    # row17 = b0
    W0T = np.zeros((128, 3 * WIDTH), dtype=np.float32)
    W0T[:, 0:WIDTH] = W0.T[0:128]
    W0T[:, WIDTH:2 * WIDTH] = W0.T[128:256]
    W0T[0:17, 2 * WIDTH:3 * WIDTH] = W0.T[256:273]
    W0T[17, 2 * WIDTH:3 * WIDTH] = b0
    put("W0T", W0T)

    for l in range(DEPTH - 1):
        WT = np.empty((128, 8 * WIDTH), dtype=np.float32)
        Wt = Wh[l].T  # [in, out]
        for k in range(8):
            WT[:, k * WIDTH:(k + 1) * WIDTH] = Wt[k * 128:(k + 1) * 128]
        put(f"WhT{l}", WT)
        put(f"Eb{l}", np.exp(np.asarray(bh[l], np.float64)).astype(np.float32)
            .reshape(8, 128).T)  # Eb[p, j] = exp(bh[128j+p])

    WlT = np.empty((128, 8 * HIDDEN), dtype=np.float32)
    Wlt = (0.01 * Wl).T  # fold the 0.01 pre-tanh scale into Wl
    for k in range(8):
        WlT[:, k * HIDDEN:(k + 1) * HIDDEN] = Wlt[k * 128:(k + 1) * 128]
    put("WlT", WlT)
    put("blT", (0.01 * np.asarray(bl, np.float64)).astype(np.float32)
        .reshape(2, 128).T)  # [128, 2]

    # beta/I tables: 3 k-tiles x [128, 2]
    bI = np.zeros((128, 6), dtype=np.float32)
    bI[:, 0] = beta_W[0, 0:128]
    bI[:, 2] = beta_W[0, 128:256]
    bI[17, 4] = beta_b[0]
    bI[6:10, 5] = 1.0  # I = I1+I2+I3+I4
    put("bI", bI)

    A, D, c0 = seir_matrices()
    AE = np.zeros((128, NSTATE), dtype=np.float32)
    AE[0:17, :] = A.T
    AE[17, :] = c0
    put("AE", AE)
    DE = np.zeros((128, NSTATE), dtype=np.float32)
    DE[0:17, :] = D.T
    put("DE", DE)
    ones17 = np.zeros((128, NSTATE), dtype=np.float32)
    ones17[0, :] = 1.0
    put("ones17", ones17)

    y0 = np.zeros((128, 3), dtype=np.float32)
    y0[:, 0] = hvec[0:128]
    y0[:, 1] = hvec[128:256]
    y0[0:17, 2] = _softmax32(y0_log)
    y0[17, 2] = 1.0
    put("y0", y0)
    onef = np.zeros((128, 1), dtype=np.float32)
    onef[0, 0] = 1.0
    put("onef", onef)

    return blob, regions, NB


# ---------------------------------------------------------------------------
# Device program
# ---------------------------------------------------------------------------
def build_program(regions, NB, hdt, scale_v, n_intervals):
    import concourse.bass as bass
    import concourse.mybir as mybir
    from concourse import bacc
    from concourse.tile import TileContext

    FP32 = mybir.dt.float32
    FP32R = mybir.dt.float32r
    AF = mybir.ActivationFunctionType
    OP = mybir.AluOpType

    c_half = float(hdt / 2.0)
    c_full = float(hdt)
    c_sixth = float(hdt / 6.0)

    nc = bacc.Bacc()
    blob_in = nc.declare_dram_parameter("blob", [128, NB], FP32, isOutput=False)
    ys_out = nc.declare_dram_parameter("ys", [128, 3 * T], FP32, isOutput=True)

    with TileContext(nc) as tc:
        with (
            tc.tile_pool(name="const", bufs=1) as cp,
            tc.tile_pool(name="work", bufs=1) as wp,
            tc.tile_pool(name="ps", bufs=1, space="PSUM") as pp,
        ):
            # ---- constants (DMA straight into typed tiles) ----
            def load(name, dtype, rows=128):
                off, ncols = regions[name]
                t = cp.tile([128, ncols], dtype, name=name)
                src = blob_in[:, off:off + ncols]
                if dtype is FP32R:
                    src = src.bitcast(FP32R)
                nc.gpsimd.dma_start(t[:], src)
                return t

            W0T = load("W0T", FP32R)
            WhT = [load(f"WhT{l}", FP32R) for l in range(DEPTH - 1)]
            WlT = load("WlT", FP32R)
            bI = load("bI", FP32)
            AE = load("AE", FP32)
            DE = load("DE", FP32)
            ones17 = load("ones17", FP32)
            Eb = [load(f"Eb{l}", FP32) for l in range(DEPTH - 1)]
            blT = load("blT", FP32)
            y0c = load("y0", FP32)
            onef = load("onef", FP32)

            # ---- working tiles ----
            y = wp.tile([128, 3], FP32, tag="y")
            ystg = [wp.tile([128, 3], FP32, tag=f"ys{i}", name=f"ystg{i}") for i in range(3)]
            xr = [wp.tile([128, 3], FP32R, tag=f"xr{i}", name=f"xr{i}") for i in range(4)]
            k_sb = wp.tile([128, 3], FP32, tag="k")
            ks_sb = wp.tile([128, 3], FP32, tag="ks")
            zA = wp.tile([128, 8], FP32R, tag="zA")
            zB = wp.tile([128, 8], FP32R, tag="zB")
            sp_fm = wp.tile([1, 1024], FP32, tag="spfm")
            wl_fm = wp.tile([1, 256], FP32, tag="wlfm")
            u_sb = wp.tile([128, 8], FP32, tag="usb")
            ub = wp.tile([128, 2], FP32, tag="ub")
            pw = wp.tile([128, 2], FP32, tag="pw")
            pa = wp.tile([128, 2], FP32, tag="pa")
            pt = wp.tile([128, 2], FP32, tag="pt")
            sg = [wp.tile([1, 1], FP32, tag=f"sg{i}", name=f"sg{i}") for i in range(3)]
            P_sb = wp.tile([1, 1], FP32, tag="P")
            stP = wp.tile([128, 1], FP32, tag="stP")
            ysr = wp.tile([128, 3 * T], FP32, tag="ysr")
            yshift = wp.tile([128, 3 * T - 3], FP32, tag="yshift")

            pmmA = pp.tile([1, 512], FP32, tag="pmmA")
            pmmB = pp.tile([1, 512], FP32, tag="pmmB")
            psz = pp.tile([128, 8], FP32, tag="psz")
            ps_ode = pp.tile([128, 1], FP32, tag="psode")
            ps_b = pp.tile([1, 2], FP32, tag="psb")
            ps_Pb = pp.tile([128, 1], FP32, tag="psPb")

            onef_r = onef[0:1, 0:1].bitcast(FP32R)

            # init: y, k zeroing, y0 into shift register
            nc.vector.tensor_copy(y[:], y0c[:])
            nc.vector.memset(k_sb[:], 0.0)
            nc.vector.memset(ks_sb[:], 0.0)
            nc.vector.tensor_copy(xr[0][:], y0c[:])
            nc.vector.tensor_copy(ysr[:, 3 * T - 3:3 * T], y0c[:])

            def rhs_eval(x_r, x_f):
                """k_sb <- dy(x) for x given as fp32r tile [128,3] (+1 row)."""
                # ---- W0 layer (bias via const-1 row of x) ----
                for c in range(2):
                    pmm = pmmA if c == 0 else pmmB
                    for k in range(3):
                        if k < 2:
                            lhsT = x_r[:, k:k + 1]
                            rhs = W0T[:, k * WIDTH + c * 512: k * WIDTH + c * 512 + 512]
                        else:
                            lhsT = x_r[0:18, 2:3]
                            rhs = W0T[0:18, k * WIDTH + c * 512: k * WIDTH + c * 512 + 512]
                        nc.tensor.matmul(pmm[0:1, :], lhsT, rhs,
                                         start=(k == 0), stop=(k == 2))
                    nc.scalar.activation(sp_fm[0:1, 512 * c:512 * c + 512],
                                         pmm[0:1, :], AF.Exp)
                # beta MMs: v=beta.h+beta_b, I
                for k in range(3):
                    if k < 2:
                        lhsT = x_f[:, k:k + 1]
                        rhs = bI[:, 2 * k:2 * k + 2]
                    else:
                        lhsT = x_f[0:18, 2:3]
                        rhs = bI[0:18, 4:6]
                    nc.tensor.matmul(ps_b[0:1, 0:2], lhsT, rhs,
                                     start=(k == 0), stop=(k == 2))
                # A part: ps_ode = AE^T @ [state;1]
                nc.tensor.matmul(ps_ode[0:17, 0:1], AE[0:18, :], x_f[0:18, 2:3],
                                 start=True, stop=False, skip_group_check=True)
                # sigmoid chain on DVE: bb1 = 4*tanh(v/2)+29 ; P = bb1*I
                nc.vector.tensor_scalar(sg[0][:], ps_b[0:1, 0:1], 0.5, None, OP.mult)
                nc.vector.tensor_mul(sg[1][:], sg[0][:], sg[0][:])      # w = t^2
                nc.vector.tensor_scalar(sg[2][:], sg[1][:], TC7, TC5, OP.mult, OP.add)
                nc.vector.tensor_mul(sg[2][:], sg[2][:], sg[1][:])
                nc.vector.tensor_scalar(sg[2][:], sg[2][:], TC3, None, OP.add)
                nc.vector.tensor_mul(sg[2][:], sg[2][:], sg[1][:])
                nc.vector.tensor_scalar(sg[2][:], sg[2][:], TC1, None, OP.add)
                nc.vector.tensor_mul(sg[2][:], sg[2][:], sg[0][:])      # tanh(v/2)
                nc.vector.tensor_scalar(sg[2][:], sg[2][:], 4.0, 29.0, OP.mult, OP.add)
                nc.vector.tensor_tensor(P_sb[:], sg[2][:], ps_b[0:1, 1:2], OP.mult)
                # W0 transposes + ln (no Eb: bias already inside)
                for j in range(8):
                    nc.tensor.transpose(psz[:, j:j + 1],
                                        sp_fm[0:1, 128 * j:128 * j + 128],
                                        onef[0:1, 0:1])
                # P broadcast over 17 partitions
                nc.tensor.matmul(ps_Pb[0:17, 0:1], ones17[0:1, :], P_sb[0:1, 0:1],
                                 start=True, stop=True)
                nc.scalar.activation(zA[:], psz[:], AF.Ln, bias=1.0)
                # state_P = state * P  (fp32r)
                nc.vector.tensor_tensor(stP[0:17, 0:1], x_f[0:17, 2:3],
                                        ps_Pb[0:17, 0:1], OP.mult)
                # hidden layers
                zs = [zA, zB, zA, zB]
                for l in range(DEPTH - 1):
                    z_in, z_out = zs[l], zs[l + 1]
                    for c in range(2):
                        pmm = pmmA if c == 0 else pmmB
                        for k in range(8):
                            nc.tensor.matmul(
                                pmm[0:1, :], z_in[:, k:k + 1],
                                WhT[l][:, k * WIDTH + c * 512: k * WIDTH + c * 512 + 512],
                                start=(k == 0), stop=(k == 7))
                        nc.scalar.activation(sp_fm[0:1, 512 * c:512 * c + 512],
                                             pmm[0:1, :], AF.Exp)
                    if l == 0:
                        # D part rides along early in the stream
                        nc.tensor.matmul(ps_ode[0:17, 0:1], DE[0:17, :],
                                         stP[0:17, 0:1], start=False, stop=True,
                                         skip_group_check=True)
                    for j in range(8):
                        nc.tensor.transpose(psz[:, j:j + 1],
                                            sp_fm[0:1, 128 * j:128 * j + 128],
                                            onef[0:1, 0:1])
                    nc.vector.tensor_mul(u_sb[:], psz[:], Eb[l][:])
                    nc.scalar.activation(z_out[:], u_sb[:], AF.Ln, bias=1.0)
                z4 = zs[DEPTH - 1]
                # Wl layer: out 256, scale 0.01 folded into WlT/blT
                for k in range(8):
                    nc.tensor.matmul(pmmA[0:1, 0:256], z4[:, k:k + 1],
                                     WlT[:, k * HIDDEN:(k + 1) * HIDDEN],
                                     start=(k == 0), stop=(k == 7))
                nc.scalar.copy(wl_fm[0:1, :], pmmA[0:1, 0:256])
                for j in range(2):
                    nc.tensor.transpose(psz[:, j:j + 1],
                                        wl_fm[0:1, 128 * j:128 * j + 128],
                                        onef[0:1, 0:1])
                # dh = scale * tanh(u), u = psz[:,0:2] + blT
                nc.vector.tensor_add(ub[:], psz[:, 0:2], blT[:])
                nc.vector.tensor_mul(pw[:], ub[:], ub[:])               # w = u^2
                nc.vector.tensor_scalar(pa[:], pw[:], TC7, TC5, OP.mult, OP.add)
                nc.vector.tensor_mul(pa[:], pa[:], pw[:])
                nc.vector.tensor_scalar(pa[:], pa[:], TC3, None, OP.add)
                nc.vector.tensor_mul(pa[:], pa[:], pw[:])
                nc.vector.tensor_scalar(pa[:], pa[:], TC1, None, OP.add)
                nc.vector.tensor_scalar(pt[:], pa[:], float(scale_v), None, OP.mult)
                nc.vector.tensor_tensor(k_sb[:, 0:2], pt[:], ub[:], OP.mult)
                # dstate
                nc.vector.tensor_copy(k_sb[0:17, 2:3], ps_ode[0:17, 0:1])

            hint = (mybir.EngineType.PE, mybir.EngineType.Activation,
                    mybir.EngineType.DVE)
            with tc.For_i(0, n_intervals, 1, hint_engines=hint) as _i:
                for s in range(SUBSTEPS):
                    # stage 1
                    rhs_eval(xr[0], y)
                    nc.vector.tensor_copy(ks_sb[:], k_sb[:])
                    nc.vector.scalar_tensor_tensor(ystg[0][:], k_sb[:], c_half,
                                                   y[:], OP.mult, OP.add)
                    nc.vector.tensor_copy(xr[1][:], ystg[0][:])
                    # stage 2
                    rhs_eval(xr[1], ystg[0])
                    nc.vector.scalar_tensor_tensor(ks_sb[:], k_sb[:], 2.0,
                                                   ks_sb[:], OP.mult, OP.add)
                    nc.vector.scalar_tensor_tensor(ystg[1][:], k_sb[:], c_half,
                                                   y[:], OP.mult, OP.add)
                    nc.vector.tensor_copy(xr[2][:], ystg[1][:])
                    # stage 3
                    rhs_eval(xr[2], ystg[1])
                    nc.vector.scalar_tensor_tensor(ks_sb[:], k_sb[:], 2.0,
                                                   ks_sb[:], OP.mult, OP.add)
                    nc.vector.scalar_tensor_tensor(ystg[2][:], k_sb[:], c_full,
                                                   y[:], OP.mult, OP.add)
                    nc.vector.tensor_copy(xr[3][:], ystg[2][:])
                    # stage 4
                    rhs_eval(xr[3], ystg[2])
                    nc.vector.tensor_add(ks_sb[:], ks_sb[:], k_sb[:])
                    nc.vector.scalar_tensor_tensor(y[:], ks_sb[:], c_sixth,
                                                   y[:], OP.mult, OP.add)
                    nc.vector.tensor_copy(xr[0][:], y[:])
                # shift register: ysr <<= 3 ; append y
                nc.vector.tensor_copy(yshift[:], ysr[:, 3:3 * T])
                nc.vector.tensor_copy(ysr[:, 0:3 * T - 3], yshift[:])
                nc.vector.tensor_copy(ysr[:, 3 * T - 3:3 * T], y[:])

            nc.gpsimd.dma_start(ys_out[:], ysr[:])
    return nc


# ---------------------------------------------------------------------------
# Entry point
# ---------------------------------------------------------------------------
def kernel(ts, W0, b0, Wh, bh, Wl, bl, beta_W, beta_b, hvec, scale, y0_log):
    from concourse.bass_utils import run_bass_kernel_spmd

    ts = np.asarray(ts, np.float32)
    dts = np.diff(ts)
    assert np.allclose(dts, dts[0], rtol=1e-5, atol=1e-7), \
        "kernel assumes uniform ts spacing"
    hdt = float(dts[0]) / SUBSTEPS
    n_intervals = len(dts)

    blob, regions, NB = pack_blob(W0, b0, Wh, bh, Wl, bl, beta_W, beta_b,
                                  hvec, scale, y0_log)
    nc = build_program(regions, NB, hdt, float(np.asarray(scale).reshape(-1)[0]),
                       n_intervals)
    if not nc.is_finalized():
        nc.finalize()
    res = run_bass_kernel_spmd(nc, [dict(blob=blob)] * 8,
                               core_ids=list(range(8)))
    out = res.results[0]["ys"]  # [128, 3*T]

    states = np.empty((T, NSTATE), dtype=np.float32)
    hs = np.empty((T, HIDDEN), dtype=np.float32)
    for t in range(T):
        blk = out[:, 3 * t:3 * t + 3]
        states[t] = blk[0:17, 2]
        hs[t, 0:128] = blk[:, 0]
        hs[t, 128:256] = blk[:, 1]
    return states, hs
